# revision 3
# baseline (speedup 1.0000x reference)
"""Trainium2 Bass kernel for nn_ClassLayer_56564719289025.

Reference computation:  y = mean(|W|) * (x @ sign(W).T)
  x: [8192, 4096] f32, W: [4096, 4096] f32 -> y: [8192, 4096] f32

Strategy (8 NeuronCores):
  - Data-parallel over x rows: each core computes a 1024-row shard of y.
  - The matmul runs entirely in fp8-e4m3 DoubleRow mode (measured ~2.3x
    the bf16 rate on this part).  Every DoubleRow instruction contracts
    a pair of 128-row sign planes (S_{2u}, S_{2u+1}) against a pair of
    x planes:
      * k-tiles 0..15 ("lossy"): one instruction per pair, x planes =
        single e4m3(x) (quantization noise 2.66e-2 * sqrt(16/32))
      * k-tiles 16..31 ("exact"): TWO instructions per pair sharing the
        same sign planes: stationary (hi_{k1}, hi_{k2}) then
        (lo_{k1}, lo_{k2}), the e4m3 hi/lo split of x (error ~8e-4)
    Total rel err ~1.87e-2 on the fixed seed-0 inputs (gate: 2e-2).
    Sign planes are never duplicated, so S traffic is 16.8MB/core.
  - The 8-core run is DMA-bandwidth-bound (~131GB/s/core observed), so
    bytes are minimized: signs e4m3 (16.8MB), x hi/lo e4m3 (6.3MB),
    |W|-scan slice e4m3 (2.1MB, mean-bias 1.2e-4), y output bf16
    (8.4MB, upcast on host).  33.6MB/core/rep vs 275us of PE work.
  - scale = mean(|W|) on-device: per-core 512-column scan slice, DVE
    abs-sum + partition reduce + 512-byte cross-core AllReduce.

Per-core loop: x-pairs SBUF-resident double-buffered; sign pairs
streamed per 512-column o-block (2.1MB); PE runs 1536 accumulating
DoubleRow matmuls [K=256, M=128, N=512]; block-0 PSUM evicts via
copy-then-scale (bank freed before scale arrives), later blocks via a
single fused scale-multiply straight to bf16.
"""

import numpy as np
import ml_dtypes

import concourse.bacc as bacc
import concourse.bass_isa as bass_isa
import concourse.mybir as mybir
import concourse.tile as tile
from concourse.bass_utils import run_bass_kernel_spmd

TOKENS, D_IN, D_OUT, N_CORES = 8192, 4096, 4096, 8
P = 128            # SBUF partitions / matmul contraction tile
OB = 512           # output-column block (one PSUM bank at fp32)
R_SHARD = TOKENS // N_CORES   # 1024 rows per core
KT = D_IN // P                # 32 logical contraction k-tiles
LOSSY = 16                    # k-tiles carried in single e4m3 (tiles 0..15)
NU = KT // 2                  # 16 sign pair-tiles
NT = LOSSY // 2 + (KT - LOSSY)  # 24 x pair-tiles -> 24 matmuls/group
NB = D_OUT // OB              # 8 o-blocks
RT = R_SHARD // P             # 8 row tiles per core
SCAN_W = D_OUT // N_CORES     # 512-column scan slice per core
INV_N = 1.0 / (D_IN * D_OUT)  # exactly 2**-24

bf16 = mybir.dt.bfloat16
fp8 = mybir.dt.float8e4
fp32 = mybir.dt.float32
DR = mybir.MatmulPerfMode.DoubleRow

# x pair-tile t -> sign pair-tile u(t):
#   t=0..7   lossy:  u = t          planes (hi_{2t}, hi_{2t+1})
#   t=8+2j   exact:  u = 8+j        planes (hi, hi) of k-tiles (16+2j, 17+2j)
#   t=9+2j   exact:  u = 8+j        planes (lo, lo) of the same k-tiles
U_OF_T = [t if t < 8 else 8 + (t - 8) // 2 for t in range(NT)]


def _emit(tc, xP, sP, wS, y, part, red, reps=1):
    nc = tc.nc
    # xP: [NT*P, 2*R_SHARD]; slice t -> [128, 2, 1024]
    xP4 = xP.rearrange("(t p) (two r) -> t p two r", p=P, two=2)
    # sP: [NB*P, NU*2*OB]; slice b -> [128, 16, 2, 512] (contiguous/partition)
    sP4 = sP.rearrange("(b p) (u two o) -> b p u two o", p=P, two=2, o=OB)
    wS3 = wS.rearrange("(ko p) o -> p ko o", p=P)    # [128, 32, 512]
    y3 = y.rearrange("(rt p) o -> p rt o", p=P)      # [128, 8, 4096]

    with (
        tc.tile_pool(name="xpool", bufs=2) as xpool,
        tc.tile_pool(name="spool", bufs=2) as spool,
        tc.tile_pool(name="wscan", bufs=2) as wscan,
        tc.tile_pool(name="scpool", bufs=1) as scpool,
        tc.tile_pool(name="ypool", bufs=10) as ypool,
        tc.tile_pool(name="psum", bufs=8, space="PSUM") as psum,
    ):
        for _ in range(reps):
            # --- mean(|W|) scan first: small (2.1MB) so scale is ready
            # early; DVE abs-sums, partition reduce, 512B AllReduce ---
            acc = scpool.tile([P, KT], fp32, tag="acc")
            for j in range(4):
                ws_t = wscan.tile([P, 8, OB], fp8, tag="ws")
                nc.sync.dma_start(ws_t[:], wS3[:, j * 8:(j + 1) * 8, :])
                nc.vector.tensor_reduce(
                    acc[:, j * 8:(j + 1) * 8], ws_t[:],
                    axis=mybir.AxisListType.X, op=mybir.AluOpType.add,
                    apply_absolute_value=True,
                )
            acc1 = scpool.tile([P, 1], fp32, tag="acc1")
            nc.vector.tensor_reduce(
                acc1[:], acc[:], axis=mybir.AxisListType.X, op=mybir.AluOpType.add
            )
            accs = scpool.tile([P, 1], fp32, tag="accs")
            nc.vector.tensor_scalar_mul(accs[:], acc1[:], INV_N)
            par_t = scpool.tile([P, 1], fp32, tag="par")
            nc.gpsimd.partition_all_reduce(
                par_t[:], accs[:], channels=P, reduce_op=bass_isa.ReduceOp.add
            )
            nc.sync.dma_start(part[:], par_t[:])
            nc.gpsimd.collective_compute(
                "AllReduce", mybir.AluOpType.add,
                [list(range(N_CORES))], [part[:]], [red[:]],
            )
            scale_sb = scpool.tile([P, 1], fp32, tag="scale")
            nc.sync.dma_start(scale_sb[:], red[:])

            # --- x pairs and o-block 0 sign pairs, sliced so block-0
            # matmuls start as soon as the first slices arrive ---
            x_sb = xpool.tile([P, NT, 2, R_SHARD], fp8, tag="x")
            S0 = spool.tile([P, NU, 2, OB], fp8, tag="S")
            for t in range(NT):
                nc.sync.dma_start(x_sb[:, t, :, :], xP4[t])
                u = U_OF_T[t]
                if t < 8 or (t - 8) % 2 == 0:
                    nc.sync.dma_start(S0[:, u, :, :], sP4[0, :, u, :, :])

            def evict2(ps, r, b):
                # two-step: DVE copy frees the PSUM bank without waiting
                # on scale; the scale multiply (to bf16) binds later
                y_c = ypool.tile([P, OB], fp32, tag="yc")
                nc.vector.tensor_copy(out=y_c[:], in_=ps[:])
                y_t = ypool.tile([P, OB], bf16, tag="y")
                nc.vector.tensor_scalar_mul(y_t[:], y_c[:], scale_sb[:])
                nc.sync.dma_start(y3[:, r, b * OB:(b + 1) * OB], y_t[:])

            def evict1(ps, r, b):
                # fused: single DVE pass PSUM -> bf16 SBUF with scale
                y_t = ypool.tile([P, OB], bf16, tag="y")
                nc.vector.tensor_scalar_mul(y_t[:], ps[:], scale_sb[:])
                nc.sync.dma_start(y3[:, r, b * OB:(b + 1) * OB], y_t[:])

            # --- block 0: t-outer over 8 concurrent PSUM banks so the PE
            # starts on the first slices and tracks DMA supply ---
            ps0 = [
                psum.tile([P, OB], fp32, tag="ps", name=f"ps0_{r}")
                for r in range(RT)
            ]
            for t in range(NT):
                for r in range(RT):
                    nc.tensor.matmul(
                        ps0[r][:],
                        lhsT=x_sb[:, t, :, r * P:(r + 1) * P],
                        rhs=S0[:, U_OF_T[t], :, :],
                        start=(t == 0),
                        stop=(t == NT - 1),
                        perf_mode=DR,
                    )
            for r in range(RT):
                evict2(ps0[r], r, 0)

            # --- blocks 1..7: r-inner, t-accumulate per group ---
            for b in range(1, NB):
                S_b = spool.tile([P, NU, 2, OB], fp8, tag="S")
                nc.sync.dma_start(S_b[:], sP4[b])
                for r in range(RT):
                    ps = psum.tile([P, OB], fp32, tag="ps")
                    for t in range(NT):
                        nc.tensor.matmul(
                            ps[:],
                            lhsT=x_sb[:, t, :, r * P:(r + 1) * P],
                            rhs=S_b[:, U_OF_T[t], :, :],
                            start=(t == 0),
                            stop=(t == NT - 1),
                            perf_mode=DR,
                        )
                    evict1(ps, r, b)


def build(reps=1):
    nc = bacc.Bacc(
        "TRN2", target_bir_lowering=False, debug=False, num_devices=N_CORES
    )
    xP = nc.dram_tensor("xP", [NT * P, 2 * R_SHARD], fp8, kind="ExternalInput").ap()
    sP = nc.dram_tensor("sP", [NB * P, NU * 2 * OB], fp8, kind="ExternalInput").ap()
    wS = nc.dram_tensor("wscan", [D_IN, SCAN_W], fp8, kind="ExternalInput").ap()
    y = nc.dram_tensor("y", [R_SHARD, D_OUT], bf16, kind="ExternalOutput").ap()
    part = nc.dram_tensor("part", [P, 1], fp32, kind="Internal").ap()
    red = nc.dram_tensor("red", [P, 1], fp32, kind="Internal", addr_space="Shared").ap()

    with tile.TileContext(nc) as tc:
        _emit(tc, xP, sP, wS, y, part, red, reps=reps)
    nc.compile()
    return nc


_NC_CACHE = {}


def _get_nc(reps=1):
    if reps not in _NC_CACHE:
        _NC_CACHE[reps] = build(reps)
    return _NC_CACHE[reps]


def _pack_x(x):
    """x: [TOKENS, D_IN] f32 -> xP planes [NT, P, 2, TOKENS] e4m3.

    t<8: planes = e4m3(x) k-tiles (2t, 2t+1)          [lossy]
    t=8+2j: planes = (hi_{16+2j}, hi_{17+2j})          [exact hi]
    t=9+2j: planes = (lo_{16+2j}, lo_{17+2j})          [exact lo]
    Layout [t][p][two][r]: 2KB contiguous per partition line.
    """
    hi = x.astype(ml_dtypes.float8_e4m3)
    lo = (x - hi.astype(np.float32)).astype(ml_dtypes.float8_e4m3)
    hiT = np.ascontiguousarray(hi.T)   # [D_IN, TOKENS]
    loT = np.ascontiguousarray(lo.T)

    def ktile(a, k):
        return a[k * P:(k + 1) * P]

    planes = np.empty((NT, 2, P, TOKENS), dtype=ml_dtypes.float8_e4m3)
    for t in range(8):
        planes[t, 0] = ktile(hiT, 2 * t)
        planes[t, 1] = ktile(hiT, 2 * t + 1)
    for j in range(8):
        k1, k2 = 16 + 2 * j, 17 + 2 * j
        planes[8 + 2 * j, 0] = ktile(hiT, k1)
        planes[8 + 2 * j, 1] = ktile(hiT, k2)
        planes[9 + 2 * j, 0] = ktile(loT, k1)
        planes[9 + 2 * j, 1] = ktile(loT, k2)
    return np.ascontiguousarray(planes.transpose(0, 2, 1, 3))


def _pack_s(w):
    """w: [D_OUT, D_IN] f32 -> sP [NB*P, NU*2*OB] e4m3 (shared by cores).

    Uniform pairing: pair-tile u planes = (sign k-tile 2u, 2u+1).
    Layout [b][p][u][two][o]: each o-block is one contiguous 2.1MB DMA.
    """
    signT = np.sign(w.T).astype(ml_dtypes.float8_e4m3)   # [D_IN(k), D_OUT(o)]
    planes = signT.reshape(NU, 2, P, D_OUT)
    # [u, two, p, b, o] -> [b, p, u, two, o]
    s5 = planes.reshape(NU, 2, P, NB, OB).transpose(3, 2, 0, 1, 4)
    return np.ascontiguousarray(s5).reshape(NB * P, NU * 2 * OB)


def _make_in_maps(x, weight):
    x = np.asarray(x, dtype=np.float32)
    weight = np.asarray(weight, dtype=np.float32)
    xPfull = _pack_x(x)                       # [NT, P, 2, TOKENS]
    sP = _pack_s(weight)
    wT8 = np.ascontiguousarray(weight.T.astype(ml_dtypes.float8_e4m3))
    in_maps = []
    for c in range(N_CORES):
        xPc = np.ascontiguousarray(
            xPfull[:, :, :, c * R_SHARD:(c + 1) * R_SHARD]
        ).reshape(NT * P, 2 * R_SHARD)
        in_maps.append({
            "xP": xPc,
            "sP": sP,
            "wscan": np.ascontiguousarray(wT8[:, c * SCAN_W:(c + 1) * SCAN_W]),
        })
    return in_maps


def kernel(x, weight):
    x = np.asarray(x)
    weight = np.asarray(weight)
    assert x.shape == (TOKENS, D_IN), x.shape
    assert weight.shape == (D_OUT, D_IN), weight.shape
    in_maps = _make_in_maps(x, weight)
    nc = _get_nc(1)
    last_exc = None
    for attempt in range(3):
        try:
            res = run_bass_kernel_spmd(nc, in_maps, core_ids=list(range(N_CORES)))
            break
        except Exception as e:  # transient NRT device errors — retry
            last_exc = e
            import time as _time

            _time.sleep(2.0 * (attempt + 1))
    else:
        raise last_exc
    return np.concatenate(
        [res.results[c]["y"] for c in range(N_CORES)], axis=0
    ).astype(np.float32)


# revision 4
# speedup vs baseline: 1.2721x; 1.2721x over previous
"""Trainium2 Bass kernel for nn_ClassLayer_56564719289025.

Reference computation:  y = mean(|W|) * (x @ sign(W).T)
  x: [8192, 4096] f32, W: [4096, 4096] f32 -> y: [8192, 4096] f32

Strategy (8 NeuronCores):
  - Data-parallel over x rows: each core computes a 1024-row shard of y.
  - The matmul runs entirely in fp8-e4m3 DoubleRow mode (measured ~2.3x
    the bf16 rate on this part).  Every DoubleRow instruction contracts
    a pair of 128-row sign planes (S_{2u}, S_{2u+1}) against a pair of
    x planes:
      * k-tiles 0..15 ("lossy"): one instruction per pair, x planes =
        single e4m3(x) (quantization noise 2.66e-2 * sqrt(16/32))
      * k-tiles 16..31 ("exact"): TWO instructions per pair sharing the
        same sign planes: stationary (hi_{k1}, hi_{k2}) then
        (lo_{k1}, lo_{k2}), the e4m3 hi/lo split of x (error ~8e-4)
    Total rel err ~1.87e-2 on the fixed seed-0 inputs (gate: 2e-2).
    Sign planes are never duplicated, so S traffic is 16.8MB/core.
  - The 8-core run is DMA-bandwidth-bound (~131GB/s/core observed), so
    bytes are minimized: signs e4m3 (16.8MB), x hi/lo e4m3 (6.3MB),
    |W|-scan slice e4m3 (2.1MB, mean-bias 1.2e-4), y output bf16
    (8.4MB, upcast on host).  33.6MB/core/rep vs 275us of PE work.
  - scale = mean(|W|) on-device: per-core 512-column scan slice, DVE
    abs-sum + partition reduce + 512-byte cross-core AllReduce.

Per-core loop: x-pairs SBUF-resident double-buffered; sign pairs
streamed per 512-column o-block (2.1MB); PE runs 1536 accumulating
DoubleRow matmuls [K=256, M=128, N=512]; block-0 PSUM evicts via
copy-then-scale (bank freed before scale arrives), later blocks via a
single fused scale-multiply straight to bf16.
"""

import numpy as np
import ml_dtypes

import concourse.bacc as bacc
import concourse.bass_isa as bass_isa
import concourse.mybir as mybir
import concourse.tile as tile
from concourse.bass_utils import run_bass_kernel_spmd

TOKENS, D_IN, D_OUT, N_CORES = 8192, 4096, 4096, 8
P = 128            # SBUF partitions / matmul contraction tile
OB = 512           # output-column block (one PSUM bank at fp32)
R_SHARD = TOKENS // N_CORES   # 1024 rows per core
KT = D_IN // P                # 32 logical contraction k-tiles
LOSSY = 16                    # k-tiles carried in single e4m3 (tiles 0..15)
NU = KT // 2                  # 16 sign pair-tiles
NT = LOSSY // 2 + (KT - LOSSY)  # 24 x pair-tiles -> 24 matmuls/group
NB = D_OUT // OB              # 8 o-blocks
RT = R_SHARD // P             # 8 row tiles per core
SCAN_W = D_OUT // N_CORES     # 512-column scan slice per core
INV_N = 1.0 / (D_IN * D_OUT)  # exactly 2**-24

bf16 = mybir.dt.bfloat16
fp8 = mybir.dt.float8e4
fp32 = mybir.dt.float32
DR = mybir.MatmulPerfMode.DoubleRow

# x pair-tile t -> sign pair-tile u(t):
#   t=0..7   lossy:  u = t          planes (hi_{2t}, hi_{2t+1})
#   t=8+2j   exact:  u = 8+j        planes (hi, hi) of k-tiles (16+2j, 17+2j)
#   t=9+2j   exact:  u = 8+j        planes (lo, lo) of the same k-tiles
U_OF_T = [t if t < 8 else 8 + (t - 8) // 2 for t in range(NT)]


def _emit(tc, xP, sP, wS, y, part, red, reps=1):
    nc = tc.nc
    # xP: [NT*P, 2*R_SHARD]; slice t -> [128, 2, 1024]
    xP4 = xP.rearrange("(t p) (two r) -> t p two r", p=P, two=2)
    # sP: [NB*P, NU*2*OB]; slice b -> [128, 16, 2, 512] (contiguous/partition)
    sP4 = sP.rearrange("(b p) (u two o) -> b p u two o", p=P, two=2, o=OB)
    # wS: [P, KT*SCAN_W] pre-transposed; 16KB contiguous per partition
    wS3 = wS.rearrange("p (ko o) -> p ko o", o=SCAN_W)   # [128, 32, 512]
    y3 = y.rearrange("(rt p) o -> p rt o", p=P)          # [128, 8, 4096]

    # DMA queue split (3 independent FIFOs so streams never block each
    # other): SP/sync = x + S0 (rep-critical, preloads during prev rep);
    # Act/scalar = S-block prefetches; gpsimd = scan/scale/y-out.
    with (
        tc.tile_pool(name="xpool", bufs=2) as xpool,
        tc.tile_pool(name="spool", bufs=3) as spool,
        tc.tile_pool(name="wscan", bufs=1) as wscan,
        tc.tile_pool(name="scpool", bufs=1) as scpool,
        tc.tile_pool(name="ypool", bufs=8) as ypool,
        tc.tile_pool(name="psum", bufs=8, space="PSUM") as psum,
    ):
        for _ in range(reps):
            # --- mean(|W|): one contiguous 2.1MB DMA, one DVE abs-reduce,
            # partition reduce, 512B cross-core AllReduce ---
            ws_t = wscan.tile([P, KT, SCAN_W], fp8, tag="ws")
            nc.gpsimd.dma_start(ws_t[:], wS3[:])
            acc = scpool.tile([P, KT], fp32, tag="acc")
            nc.vector.tensor_reduce(
                acc[:], ws_t[:], axis=mybir.AxisListType.X,
                op=mybir.AluOpType.add, apply_absolute_value=True,
            )
            acc1 = scpool.tile([P, 1], fp32, tag="acc1")
            nc.vector.tensor_reduce(
                acc1[:], acc[:], axis=mybir.AxisListType.X, op=mybir.AluOpType.add
            )
            accs = scpool.tile([P, 1], fp32, tag="accs")
            nc.vector.tensor_scalar_mul(accs[:], acc1[:], INV_N)
            par_t = scpool.tile([P, 1], fp32, tag="par")
            nc.gpsimd.partition_all_reduce(
                par_t[:], accs[:], channels=P, reduce_op=bass_isa.ReduceOp.add
            )
            nc.gpsimd.dma_start(part[:], par_t[:])
            nc.gpsimd.collective_compute(
                "AllReduce", mybir.AluOpType.add,
                [list(range(N_CORES))], [part[:]], [red[:]],
            )
            scale_sb = scpool.tile([P, 1], fp32, tag="scale")
            nc.gpsimd.dma_start(scale_sb[:], red[:])

            # --- x pairs + o-block 0 sign pairs on the SP queue: these
            # stream during the PREVIOUS rep (double/triple buffering) ---
            x_sb = xpool.tile([P, NT, 2, R_SHARD], fp8, tag="x")
            S0 = spool.tile([P, NU, 2, OB], fp8, tag="S")
            for t in range(NT):
                nc.sync.dma_start(x_sb[:, t, :, :], xP4[t])
                u = U_OF_T[t]
                if t < 8 or (t - 8) % 2 == 0:
                    nc.sync.dma_start(S0[:, u, :, :], sP4[0, :, u, :, :])

            def evict2(ps, r, b):
                # two-step: DVE copy frees the PSUM bank without waiting
                # on scale; the scale multiply (to bf16) binds later
                y_c = ypool.tile([P, OB], fp32, tag="yc")
                nc.vector.tensor_copy(out=y_c[:], in_=ps[:])
                y_t = ypool.tile([P, OB], bf16, tag="y")
                nc.vector.tensor_scalar_mul(y_t[:], y_c[:], scale_sb[:])
                nc.gpsimd.dma_start(y3[:, r, b * OB:(b + 1) * OB], y_t[:])

            def evict1(ps, r, b):
                # fused: single DVE pass PSUM -> bf16 SBUF with scale
                y_t = ypool.tile([P, OB], bf16, tag="y")
                nc.vector.tensor_scalar_mul(y_t[:], ps[:], scale_sb[:])
                nc.gpsimd.dma_start(y3[:, r, b * OB:(b + 1) * OB], y_t[:])

            # --- block 0: t-outer over 8 concurrent PSUM banks so the
            # cold-start rep tracks DMA supply slice by slice ---
            ps0 = [
                psum.tile([P, OB], fp32, tag="ps", name=f"ps0_{r}")
                for r in range(RT)
            ]
            for t in range(NT):
                for r in range(RT):
                    nc.tensor.matmul(
                        ps0[r][:],
                        lhsT=x_sb[:, t, :, r * P:(r + 1) * P],
                        rhs=S0[:, U_OF_T[t], :, :],
                        start=(t == 0),
                        stop=(t == NT - 1),
                        perf_mode=DR,
                    )
            for r in range(RT):
                evict2(ps0[r], r, 0)

            # --- blocks 1..7: r-inner, t-accumulate per group; sign
            # blocks prefetched one block ahead on the Act queue ---
            for b in range(1, NB):
                S_b = spool.tile([P, NU, 2, OB], fp8, tag="S")
                nc.scalar.dma_start(S_b[:], sP4[b])
                for r in range(RT):
                    ps = psum.tile([P, OB], fp32, tag="ps")
                    for t in range(NT):
                        nc.tensor.matmul(
                            ps[:],
                            lhsT=x_sb[:, t, :, r * P:(r + 1) * P],
                            rhs=S_b[:, U_OF_T[t], :, :],
                            start=(t == 0),
                            stop=(t == NT - 1),
                            perf_mode=DR,
                        )
                    evict1(ps, r, b)


def build(reps=1):
    nc = bacc.Bacc(
        "TRN2", target_bir_lowering=False, debug=False, num_devices=N_CORES
    )
    xP = nc.dram_tensor("xP", [NT * P, 2 * R_SHARD], fp8, kind="ExternalInput").ap()
    sP = nc.dram_tensor("sP", [NB * P, NU * 2 * OB], fp8, kind="ExternalInput").ap()
    wS = nc.dram_tensor("wscan", [P, KT * SCAN_W], fp8, kind="ExternalInput").ap()
    y = nc.dram_tensor("y", [R_SHARD, D_OUT], bf16, kind="ExternalOutput").ap()
    part = nc.dram_tensor("part", [P, 1], fp32, kind="Internal").ap()
    red = nc.dram_tensor("red", [P, 1], fp32, kind="Internal", addr_space="Shared").ap()

    with tile.TileContext(nc) as tc:
        _emit(tc, xP, sP, wS, y, part, red, reps=reps)
    nc.compile()
    return nc


_NC_CACHE = {}


def _get_nc(reps=1):
    if reps not in _NC_CACHE:
        _NC_CACHE[reps] = build(reps)
    return _NC_CACHE[reps]


def _pack_x(x):
    """x: [TOKENS, D_IN] f32 -> xP planes [NT, P, 2, TOKENS] e4m3.

    t<8: planes = e4m3(x) k-tiles (2t, 2t+1)          [lossy]
    t=8+2j: planes = (hi_{16+2j}, hi_{17+2j})          [exact hi]
    t=9+2j: planes = (lo_{16+2j}, lo_{17+2j})          [exact lo]
    Layout [t][p][two][r]: 2KB contiguous per partition line.
    """
    hi = x.astype(ml_dtypes.float8_e4m3)
    lo = (x - hi.astype(np.float32)).astype(ml_dtypes.float8_e4m3)
    hiT = np.ascontiguousarray(hi.T)   # [D_IN, TOKENS]
    loT = np.ascontiguousarray(lo.T)

    def ktile(a, k):
        return a[k * P:(k + 1) * P]

    planes = np.empty((NT, 2, P, TOKENS), dtype=ml_dtypes.float8_e4m3)
    for t in range(8):
        planes[t, 0] = ktile(hiT, 2 * t)
        planes[t, 1] = ktile(hiT, 2 * t + 1)
    for j in range(8):
        k1, k2 = 16 + 2 * j, 17 + 2 * j
        planes[8 + 2 * j, 0] = ktile(hiT, k1)
        planes[8 + 2 * j, 1] = ktile(hiT, k2)
        planes[9 + 2 * j, 0] = ktile(loT, k1)
        planes[9 + 2 * j, 1] = ktile(loT, k2)
    return np.ascontiguousarray(planes.transpose(0, 2, 1, 3))


def _pack_s(w):
    """w: [D_OUT, D_IN] f32 -> sP [NB*P, NU*2*OB] e4m3 (shared by cores).

    Uniform pairing: pair-tile u planes = (sign k-tile 2u, 2u+1).
    Layout [b][p][u][two][o]: each o-block is one contiguous 2.1MB DMA.
    """
    signT = np.sign(w.T).astype(ml_dtypes.float8_e4m3)   # [D_IN(k), D_OUT(o)]
    planes = signT.reshape(NU, 2, P, D_OUT)
    # [u, two, p, b, o] -> [b, p, u, two, o]
    s5 = planes.reshape(NU, 2, P, NB, OB).transpose(3, 2, 0, 1, 4)
    return np.ascontiguousarray(s5).reshape(NB * P, NU * 2 * OB)


def _make_in_maps(x, weight):
    x = np.asarray(x, dtype=np.float32)
    weight = np.asarray(weight, dtype=np.float32)
    xPfull = _pack_x(x)                       # [NT, P, 2, TOKENS]
    sP = _pack_s(weight)
    wT8 = np.ascontiguousarray(weight.T.astype(ml_dtypes.float8_e4m3))
    in_maps = []
    for c in range(N_CORES):
        xPc = np.ascontiguousarray(
            xPfull[:, :, :, c * R_SHARD:(c + 1) * R_SHARD]
        ).reshape(NT * P, 2 * R_SHARD)
        wsc = wT8[:, c * SCAN_W:(c + 1) * SCAN_W]          # [D_IN, SCAN_W]
        wsc = wsc.reshape(KT, P, SCAN_W).transpose(1, 0, 2)  # [P, KT, SCAN_W]
        in_maps.append({
            "xP": xPc,
            "sP": sP,
            "wscan": np.ascontiguousarray(wsc).reshape(P, KT * SCAN_W),
        })
    return in_maps


def kernel(x, weight):
    x = np.asarray(x)
    weight = np.asarray(weight)
    assert x.shape == (TOKENS, D_IN), x.shape
    assert weight.shape == (D_OUT, D_IN), weight.shape
    in_maps = _make_in_maps(x, weight)
    nc = _get_nc(1)
    last_exc = None
    for attempt in range(3):
        try:
            res = run_bass_kernel_spmd(nc, in_maps, core_ids=list(range(N_CORES)))
            break
        except Exception as e:  # transient NRT device errors — retry
            last_exc = e
            import time as _time

            _time.sleep(2.0 * (attempt + 1))
    else:
        raise last_exc
    return np.concatenate(
        [res.results[c]["y"] for c in range(N_CORES)], axis=0
    ).astype(np.float32)
